# revision 4
# baseline (speedup 1.0000x reference)
"""MinGRU (2-layer) Trainium2 Bass kernel — fp8 DoubleRow matmul version.

Problem: h[8,4096,1024] f32, W0/W1 [1024,3072] f32.
Per layer: z = h @ W; hidden,gate,proj = split(z);
  a = sigmoid(-gate); gh = relu(hidden) + min(sigmoid(hidden), 0.5)
  scan: out_t = a_t*out_{t-1} + (1-a_t)*gh_t   (DVE scan, fp32 state)
  h' = h + sigmoid(proj)*(out - h)

Sharding: batch row per core (B=8 over 8 cores), weights replicated.

Layout: host pre-transposes h to [H, T] and supplies BOTH fp8(e4m3) and
fp16 copies; weights are host-quantized to fp8 and DoubleRow-interleaved
([2, NK/2, 128, 2, 3H]: two k-subtiles per matmul instruction). Output is
written [H, T] fp16 and un-transposed + upcast on host.

Engine split per [128,512] tile (128 tiles total):
  PE:   12 fp8 DoubleRow matmuls (256-contract each) -> ph/pg/pp PSUM
  ACT:  3 sigmoids (s_, a_, g_) fp16 out; L1-only fp16->fp8 cast of h1
  DVE:  smin -> gh(stt) -> negb(stt) -> scan -> carry -> d -> m -> h' add
  DMA:  plain strided loads/stores, no transposes (host pre/post-transposes)
"""

import sys

if "/opt/trn_rl_repo" not in sys.path:
    sys.path.insert(0, "/opt/trn_rl_repo")


from contextlib import ExitStack

import numpy as np
import ml_dtypes

import concourse.bass as bass
import concourse.tile as tile
from concourse import bacc, mybir
from concourse import bass_utils

T, H, H3 = 4096, 1024, 3072
TC = 512                 # time chunk (= one PSUM bank of fp32)
NCHUNK = T // TC
NFB = H // 128           # feature blocks (partition tiles)
NK = H // 128            # contraction tiles
NJ2 = NK // 2            # DoubleRow k-pairs
F32 = mybir.dt.float32
F16 = mybir.dt.float16
F8 = mybir.dt.float8e4
ACT = mybir.ActivationFunctionType
ALU = mybir.AluOpType
DR = mybir.MatmulPerfMode.DoubleRow


TAIL_LAG = 2  # calls between a tile's head ops and its tail (h'/cast/DMA-out)


def _emit_body(tc, y16, h8t, h16t, w_sb, pools):
    """One full forward pass (both layers, all chunks) for this core.

    Feature blocks are processed in PAIRS: ACT writes sigmoid outputs into
    halves of shared [128,2,TC] tiles and every SBUF-side DVE op (negb, d,
    m, highway add) runs once per pair at [128,1024], halving the DVE/ACT
    instruction count per tile -- the measured bottleneck is per-call
    instruction/semaphore structure, not engine throughput. Only the scans
    and the PSUM-reading ops (sigmoids, gh) stay per-f. Pair tails (highway
    add + cast/output-DMA) are emitted one pair late so in-order queues
    never wait on a cross-engine round trip.
    """
    nc = tc.nc
    io_pool, h1_pool, mm_psum, ew, carry_pool = pools

    carries = carry_pool.tile([128, 2 * NFB], F32)
    y16r = y16.rearrange("(fb p) t -> p fb t", p=128)
    pending = []

    def flush(limit):
        while len(pending) > limit:
            pending.pop(0)()

    def emit_pair(i, li, fp, rhs8, rhs16, h1T16, h1T8):
        f0 = 2 * fp
        s_p = ew.tile([128, 2, TC], F16, tag="s")
        a_p = ew.tile([128, 2, TC], F16, tag="a")
        g_p = ew.tile([128, 2, TC], F16, tag="g", bufs=3)
        gh_p = ew.tile([128, 2, TC], F16, tag="gh")
        sc_p = ew.tile([128, 2, TC], F16, tag="sc")
        for q in (0, 1):
            f = f0 + q
            ph = mm_psum.tile([128, TC], F32, tag="ph")
            pg = mm_psum.tile([128, TC], F32, tag="pg")
            pp = mm_psum.tile([128, TC], F32, tag="pp")
            for j in range(NJ2):
                st = dict(start=(j == 0), stop=(j == NJ2 - 1), perf_mode=DR)
                lw = w_sb[li * NJ2 + j]
                rr = rhs8[:, 2 * j:2 * j + 2, :]
                nc.tensor.matmul(ph[:], lw[:, :, f * 128:(f + 1) * 128], rr, **st)
                nc.tensor.matmul(pg[:], lw[:, :, H + f * 128:H + (f + 1) * 128],
                                 rr, **st)
                nc.tensor.matmul(pp[:], lw[:, :, 2 * H + f * 128:2 * H + (f + 1) * 128],
                                 rr, **st)
            nc.scalar.activation(s_p[:, q, :], ph[:], ACT.Sigmoid)
            nc.scalar.activation(a_p[:, q, :], pg[:], ACT.Sigmoid, scale=-1.0)
            nc.scalar.activation(g_p[:, q, :], pp[:], ACT.Sigmoid)
            smin = ew.tile([128, TC], F16, tag="smin")
            nc.vector.tensor_scalar_min(smin[:], s_p[:, q, :], 0.5)
            nc.vector.scalar_tensor_tensor(
                gh_p[:, q, :], ph[:], 0.0, smin[:], op0=ALU.max, op1=ALU.add)
        # ---- paired SBUF-side chain ----
        negb_p = ew.tile([128, 2, TC], F16, tag="negb")
        nc.vector.scalar_tensor_tensor(
            negb_p[:], a_p[:], 1.0, gh_p[:], op0=ALU.subtract, op1=ALU.mult)
        col = li * NFB + f0
        for q in (0, 1):
            init = 0.0 if i == 0 else carries[:, col + q:col + q + 1]
            nc.vector.tensor_tensor_scan(
                sc_p[:, q, :], a_p[:, q, :], negb_p[:, q, :], init,
                op0=ALU.mult, op1=ALU.subtract)
        if i < NCHUNK - 1:
            nc.vector.tensor_copy(carries[:, col:col + 2], sc_p[:, :, TC - 1:TC])
        rr16 = rhs16[:, f0:f0 + 2, :]
        d_p = ew.tile([128, 2, TC], F16, tag="d")
        nc.vector.tensor_tensor(d_p[:], sc_p[:], rr16, op=ALU.subtract)
        m_p = ew.tile([128, 2, TC], F16, tag="m", bufs=4)
        nc.vector.tensor_tensor(m_p[:], g_p[:], d_p[:], op=ALU.mult)

        def tail():
            if li == 0:
                nc.vector.tensor_tensor(h1T16[:, f0:f0 + 2, :], m_p[:], rr16,
                                        op=ALU.add)
                nc.scalar.copy(h1T8[:, f0:f0 + 2, :], h1T16[:, f0:f0 + 2, :])
            else:
                yo = ew.tile([128, 2, TC], F16, tag="yo", bufs=4)
                nc.vector.tensor_tensor(yo[:], m_p[:], rr16, op=ALU.add)
                nc.sync.dma_start(
                    y16r[:, f0:f0 + 2, i * TC:(i + 1) * TC], yo[:])

        pending.append(tail)
        flush(1)

    def emit_layer(i, li, rhs8, rhs16, h1T16, h1T8):
        for fp in range(NFB // 2):
            emit_pair(i, li, fp, rhs8, rhs16, h1T16, h1T8)

    # Layer-2 runs one chunk behind layer-1 so its input (h1) is complete.
    prev = None
    for i in range(NCHUNK):
        hT8 = io_pool.tile([128, NK, TC], F8, tag="h8")
        nc.sync.dma_start(
            hT8[:],
            h8t.rearrange("(k p) t -> p k t", p=128)[:, :, i * TC:(i + 1) * TC])
        hT16 = io_pool.tile([128, NK, TC], F16, tag="h16")
        nc.sync.dma_start(
            hT16[:],
            h16t.rearrange("(k p) t -> p k t", p=128)[:, :, i * TC:(i + 1) * TC])
        h1T16 = h1_pool.tile([128, NK, TC], F16, tag="h1_16")
        h1T8 = h1_pool.tile([128, NK, TC], F8, tag="h1_8")
        emit_layer(i, 0, hT8, hT16, h1T16, h1T8)
        if prev is not None:
            emit_layer(i - 1, 1, prev[1], prev[0], None, None)
        prev = (h1T16, h1T8)
    emit_layer(NCHUNK - 1, 1, prev[1], prev[0], None, None)
    flush(0)


def build_nc(loop_iters: int = 1):
    """Build + compile the per-core Bass program (SPMD across 8 cores)."""
    nc = bacc.Bacc("TRN2", target_bir_lowering=False, debug=False,
                   enable_asserts=False, num_devices=8)
    h8t = nc.dram_tensor("h8t", [H, T], F8, kind="ExternalInput").ap()
    h16t = nc.dram_tensor("h16t", [H, T], F16, kind="ExternalInput").ap()
    w8 = nc.dram_tensor("w8", [2, NJ2, 128, 2, H3], F8, kind="ExternalInput").ap()
    iden = nc.dram_tensor("iden", [128, 128], F16, kind="ExternalInput").ap()
    y16 = nc.dram_tensor("y16", [H, T], F16, kind="ExternalOutput").ap()

    with tile.TileContext(nc) as tc:
        with ExitStack() as ctx:
            wpool = ctx.enter_context(tc.tile_pool(name="w", bufs=1))
            const = ctx.enter_context(tc.tile_pool(name="const", bufs=1))
            io_pool = ctx.enter_context(tc.tile_pool(name="io", bufs=2))
            h1_pool = ctx.enter_context(tc.tile_pool(name="h1", bufs=2))
            mm_psum = ctx.enter_context(
                tc.tile_pool(name="mmp", bufs=2, space="PSUM"))
            ew = ctx.enter_context(tc.tile_pool(name="ew", bufs=2))
            carry_pool = ctx.enter_context(tc.tile_pool(name="carry", bufs=1))

            w_sb = [wpool.tile([128, 2, H3], F8, name=f"w{li}_{j}", tag=f"w{li}_{j}")
                    for li in range(2) for j in range(NJ2)]
            for li in range(2):
                for j in range(NJ2):
                    nc.sync.dma_start(w_sb[li * NJ2 + j][:], w8[li, j])
            iden_sb = const.tile([128, 128], F16)
            nc.sync.dma_start(iden_sb[:], iden[:])
            # PE clock (HAM) warmup + ACT sigmoid-table preload while the
            # weight stream is in flight.
            warm_ps = mm_psum.tile([128, TC], F32, tag="ph")
            for _ in range(16):
                nc.tensor.matmul(warm_ps[:, 0:128], iden_sb[:], iden_sb[:],
                                 start=True, stop=True)
            warm_sb = ew.tile([128, TC], F16, tag="s")
            nc.scalar.activation(warm_sb[:, 0:1], warm_ps[:, 0:1], ACT.Sigmoid)

            pools = (io_pool, h1_pool, mm_psum, ew, carry_pool)
            if loop_iters == 1:
                _emit_body(tc, y16, h8t, h16t, w_sb, pools)
            else:
                with tc.For_i(0, loop_iters, 1):
                    _emit_body(tc, y16, h8t, h16t, w_sb, pools)
    nc.compile()
    return nc


_CACHED_NC = None


def _prep_inputs(h, W0, W1):
    W = np.stack([np.asarray(W0), np.asarray(W1)])          # [2, H, 3H]
    w8 = np.ascontiguousarray(
        W.reshape(2, NJ2, 2, 128, H3).transpose(0, 1, 3, 2, 4)
    ).astype(ml_dtypes.float8_e4m3)                          # [2, NJ2, 128, 2, 3H]
    iden = np.eye(128, dtype=np.float16)
    maps = []
    for c in range(8):
        hT = np.ascontiguousarray(np.asarray(h[c]).T)        # [H, T] f32
        maps.append({
            "h8t": hT.astype(ml_dtypes.float8_e4m3),
            "h16t": hT.astype(np.float16),
            "w8": w8,
            "iden": iden,
        })
    return maps


def kernel(h, W0, W1):
    global _CACHED_NC
    if _CACHED_NC is None:
        _CACHED_NC = build_nc()
    res = bass_utils.run_bass_kernel_spmd(
        _CACHED_NC, _prep_inputs(h, W0, W1), core_ids=list(range(8)))
    return np.stack(
        [res.results[c]["y16"].astype(np.float32).T for c in range(8)], axis=0)
